# revision 11
# baseline (speedup 1.0000x reference)
"""Trainium2 Bass kernel for nn_CustomDense (bit-serial quantized dense layer).

Math: the reference's per-element bit-serial shift-add loop computes exactly
    f(x, w) = trunc(x * w / 256)          (bits=8, x in [0,15], w in [-128,127])
so  out = relu(sum_j f(x_ij, w_ju) + bias_u).

Device algorithm (exact, integer-precise):
  one-hot over the 15 nonzero activation values v:
      sum_j f = sum_v (X==v) @ floor(v*W/256) + (trunc - floor) correction.

  G_v is produced in ONE dve op per v via a magic-number trick: the DVE
  computes z = w*(v/256) + (1536 - 511/1024) in fp32 and writes fp16; fp16
  spacing is exactly 1.0 on [1024, 2048), and z is never a tie (4m-511 is
  odd), so round-to-nearest-fp16 gives exactly 1536 + floor(v*w/256).
  The spurious +1536 per product sums to 1536*nnz(x_i), cancelled exactly by
  the Xnz @ (Mneg - 1536) group (values -1536/-1535 are fp16-exact).

  trunc-floor correction: trunc = floor + 1[w<0 and x*|w| % 256 != 0]:
      + Xnz@Mneg - XE1@D128 - XE2@(D64+D128) - XE3@(D32+D64+D96+D128)
  with XE1=H2+H6+H10+H14, XE2=H4+H12, XE3=H8 (sums of existing one-hot
  masks), Dm=1[w=-m] (negated on-chip).

All matmul operands are fp16 (0/1 masks, small ints, 1536+-8: all exact);
PSUM accumulates in fp32 and every partial sum stays < 2^24, so the whole
pipeline is integer-exact.

Sharding: D (contraction, 1024) split across 8 cores, 128 rows each; every
core computes a full [64, 1024] partial in PSUM. Host sums the 8 partials
(exact), adds bias in fp32 and applies relu -- bit-identical to the
reference.
"""

import numpy as np

B, D, U, BITS = 64, 1024, 1024, 8
NCORES = 8
DSH = D // NCORES  # 128 contraction rows per core
MAGIC = 1536.0
OFF = MAGIC - 511.0 / 1024.0

# engine for each G_v pass: "dve" or "act"
G_ENGINE = {v: ("act" if v in (13, 14, 15) else "dve") for v in range(1, 16)}
N_WARMUP_MM = 6  # dummy matmuls during the DMA-in window to warm the PE HAM
TRACE = False

_NC_CACHE = {}


def _build_nc():
    import concourse.bacc as bacc
    import concourse.mybir as mybir
    import concourse.tile as tile

    Alu = mybir.AluOpType
    f16 = mybir.dt.float16
    i16 = mybir.dt.int16
    f32 = mybir.dt.float32

    nc = bacc.Bacc("TRN2", target_bir_lowering=False, debug=False)
    xt_d = nc.dram_tensor("xt", [DSH, B], i16, kind="ExternalInput")
    w_d = nc.dram_tensor("w", [DSH, U], i16, kind="ExternalInput")
    out_d = nc.dram_tensor("out", [B, U], f32, kind="ExternalOutput")

    with tile.TileContext(nc) as tc:
        with (
            tc.tile_pool(name="io", bufs=1) as io,
            tc.tile_pool(name="ps", bufs=1, space="PSUM") as ps,
        ):
            xt_sb = io.tile([DSH, B], i16)
            w_sb = io.tile([DSH, U], i16)
            nc.sync.dma_start(xt_sb[:], xt_d[:])
            nc.sync.dma_start(w_sb[:], w_d[:])

            # --- PE warmup: dummy matmuls on memset tiles during DMA-in ---
            warm_l = io.tile([DSH, B], f16, tag="warm_l")
            warm_r = io.tile([DSH, 512], f16, tag="warm_r")
            nc.gpsimd.memset(warm_l[:], 1.0)
            nc.gpsimd.memset(warm_r[:], 1.0)
            off_sb = io.tile([DSH, 1], f32, tag="offsb")
            nc.gpsimd.memset(off_sb[:], OFF)
            warm_ps = ps.tile([B, 512], f32, tag="warm_ps")
            for _ in range(N_WARMUP_MM):
                nc.tensor.matmul(
                    warm_ps[:], warm_l[:], warm_r[:], start=True, stop=True
                )

            # --- per-v: one-hot mask (fp16 [DSH, B]), LUT, 2 matmuls ---
            # G_v = 1536 + floor(v*W/256), fp16, one op per v
            acc = ps.tile([B, U], f32)
            h, g = {}, {}
            mm_queue = []  # (lhsT, rhs_tile, half) in PE issue order

            def emit_hv(v):
                t = io.tile([DSH, B], f16, tag=f"h{v}")
                nc.vector.tensor_scalar(
                    out=t[:], in0=xt_sb[:], scalar1=float(v), scalar2=None,
                    op0=Alu.is_equal,
                )
                h[v] = t

            def emit_gv(v):
                t = io.tile([DSH, U], f16, tag=f"g{v}")
                if G_ENGINE[v] == "act":
                    nc.scalar.activation(
                        t[:], w_sb[:], mybir.ActivationFunctionType.Identity,
                        bias=off_sb[:], scale=float(v) / 256.0,
                    )
                else:
                    nc.vector.tensor_scalar(
                        out=t[:], in0=w_sb[:], scalar1=float(v) / 256.0,
                        scalar2=OFF, op0=Alu.mult, op1=Alu.add,
                    )
                g[v] = t

            # --- W-side correction masks on GPSIMD (parallel engine) ---
            mneg = io.tile([DSH, U], f16, tag="mneg")
            nc.gpsimd.tensor_scalar(
                out=mneg[:], in0=w_sb[:], scalar1=0.0, scalar2=-MAGIC,
                op0=Alu.is_lt, op1=Alu.add,
            )
            dmask = {}
            for m in (128, 64, 32, 96):
                t = io.tile([DSH, U], f16, tag=f"d{m}")
                nc.gpsimd.tensor_scalar(
                    out=t[:], in0=w_sb[:], scalar1=float(-m),
                    scalar2=-1.0, op0=Alu.is_equal, op1=Alu.mult,
                )
                dmask[m] = t

            # ACT-assigned G's first (ACT is otherwise idle; starts at W-DMA)
            for v in range(1, 16):
                if G_ENGINE[v] == "act":
                    emit_gv(v)

            # xnz early on DVE (tiny)
            xnz = io.tile([DSH, B], f16, tag="xnz")
            nc.vector.tensor_scalar(
                out=xnz[:], in0=xt_sb[:], scalar1=1.0, scalar2=None,
                op0=Alu.is_ge,
            )
            for v in range(1, 16):
                emit_hv(v)
                if G_ENGINE[v] != "act":
                    emit_gv(v)
            # XE sums of one-hot masks (disjoint -> still 0/1) + D-mask sums
            xe1 = io.tile([DSH, B], f16, tag="xe1")
            xe1b = io.tile([DSH, B], f16, tag="xe1b")
            xe2 = io.tile([DSH, B], f16, tag="xe2")
            nc.vector.tensor_tensor(
                out=xe1b[:], in0=h[2][:], in1=h[6][:], op=Alu.add
            )
            nc.vector.tensor_tensor(
                out=xe1[:], in0=h[10][:], in1=h[14][:], op=Alu.add
            )
            nc.vector.tensor_tensor(
                out=xe1[:], in0=xe1[:], in1=xe1b[:], op=Alu.add
            )
            nc.vector.tensor_tensor(
                out=xe2[:], in0=h[4][:], in1=h[12][:], op=Alu.add
            )
            p2n = io.tile([DSH, U], f16, tag="p2n")
            nc.vector.tensor_tensor(
                out=p2n[:], in0=dmask[64][:], in1=dmask[128][:], op=Alu.add
            )
            p3n = io.tile([DSH, U], f16, tag="p3n")
            nc.vector.tensor_tensor(
                out=p3n[:], in0=dmask[32][:], in1=dmask[96][:], op=Alu.add
            )
            nc.vector.tensor_tensor(
                out=p3n[:], in0=p3n[:], in1=p2n[:], op=Alu.add
            )

            # --- matmuls: 19 groups x 2 halves; psum [64, 1024] = 2 banks ---
            # v-major pairs for v=1..12 (PE chases DVE), then the remaining
            # 7 groups half-major so bank 0's stop lands ~7 MMs early and
            # its copy+DMA overlap bank 1's matmuls.
            groups = [(h[v], g[v]) for v in range(1, 16)] + [
                (xnz, mneg), (xe1, dmask[128]), (xe2, p2n), (h[8], p3n)]
            n_g = len(groups)
            sched = []  # (gidx, half)
            for gidx in range(12):
                sched += [(gidx, 0), (gidx, 1)]
            sched += [(gidx, 0) for gidx in range(12, n_g)]
            sched += [(gidx, 1) for gidx in range(12, n_g)]
            for gidx, half in sched:
                lhsT, rhs = groups[gidx]
                sl = slice(half * 512, (half + 1) * 512)
                nc.tensor.matmul(
                    acc[:, sl], lhsT[:], rhs[:, sl],
                    start=(gidx == 0), stop=(gidx == n_g - 1),
                )

            # --- epilogue: PSUM -> SBUF fp32 per half, 2 DMAs out ---
            o_sb = io.tile([B, U], f32, tag="osb")
            nc.scalar.copy(o_sb[:, 0:512], acc[:, 0:512])
            nc.sync.dma_start(out_d[:, 0:512], o_sb[:, 0:512])
            nc.scalar.copy(o_sb[:, 512:768], acc[:, 512:768])
            nc.vector.tensor_copy(o_sb[:, 768:1024], acc[:, 768:1024])
            nc.sync.dma_start(out_d[:, 512:1024], o_sb[:, 512:1024])

    nc.compile()
    return nc


def _get_nc():
    if "nc" not in _NC_CACHE:
        _NC_CACHE["nc"] = _build_nc()
    return _NC_CACHE["nc"]


_LAST_RESULTS = {}


def _kernel_numpy(inputs, bits, kernel, bias):
    # generic (non-8-bit) fallback; mirrors the reference exactly
    x = np.asarray(inputs, np.float64)
    w = np.asarray(kernel, np.float64)
    b = int(bits)
    out = np.zeros((x.shape[0], w.shape[1]), np.float64)
    scale = float(2 ** b)
    for d0 in range(0, w.shape[0], 128):
        d1 = min(d0 + 128, w.shape[0])
        wm = np.sign(w[None, d0:d1, :]) * (
            np.abs(w[None, d0:d1, :]) % scale if b < 31 else np.abs(w[None, d0:d1, :])
        )
        out += np.trunc(x[:, d0:d1, None] * wm / scale).sum(1)
    return np.maximum(out + np.asarray(bias, np.float64)[None, :], 0.0).astype(
        np.float32
    )


def kernel(inputs, bits, kernel, bias):
    if int(bits) != BITS:
        return _kernel_numpy(inputs, bits, kernel, bias)

    from concourse.bass_utils import run_bass_kernel_spmd

    x = np.asarray(inputs)
    w = np.asarray(kernel)
    b = np.asarray(bias, dtype=np.float32)
    assert x.shape == (B, D) and w.shape == (D, U)

    xt = np.ascontiguousarray(x.T.astype(np.int16))  # [D, B]
    wi = np.ascontiguousarray(w.astype(np.int16))    # [D, U]

    in_maps = [
        {
            "xt": np.ascontiguousarray(xt[c * DSH:(c + 1) * DSH]),
            "w": np.ascontiguousarray(wi[c * DSH:(c + 1) * DSH]),
        }
        for c in range(NCORES)
    ]

    nc = _get_nc()
    res = run_bass_kernel_spmd(
        nc, in_maps, core_ids=list(range(NCORES)), trace=TRACE
    )
    _LAST_RESULTS["res"] = res

    total = np.zeros((B, U), dtype=np.float32)
    for r in res.results:
        total += r["out"]
    return np.maximum(total + b[None, :], 0.0).astype(np.float32)


# revision 12
# speedup vs baseline: 3.5762x; 3.5762x over previous
"""Trainium2 Bass kernel for nn_CustomDense (bit-serial quantized dense layer).

Math: the reference's per-element bit-serial shift-add loop computes exactly
    f(x, w) = trunc(x * w / 256)          (bits=8, x in [0,15], w in [-128,127])
so  out = relu(sum_j f(x_ij, w_ju) + bias_u).

Device algorithm (exact, integer-precise):
  one-hot over the 15 nonzero activation values v:
      sum_j f = sum_v (X==v) @ floor(v*W/256) + (trunc - floor) correction.

  G_v is produced in ONE dve op per v via a magic-number trick: the DVE
  computes z = w*(v/256) + (1536 - 511/1024) in fp32 and writes fp16; fp16
  spacing is exactly 1.0 on [1024, 2048), and z is never a tie (4m-511 is
  odd), so round-to-nearest-fp16 gives exactly 1536 + floor(v*w/256).
  The spurious +1536 per product sums to 1536*nnz(x_i), cancelled exactly by
  the Xnz @ (Mneg - 1536) group (values -1536/-1535 are fp16-exact).

  trunc-floor correction: trunc = floor + 1[w<0 and x*|w| % 256 != 0]:
      + Xnz@Mneg - XE1@D128 - XE2@(D64+D128) - XE3@(D32+D64+D96+D128)
  with XE1=H2+H6+H10+H14, XE2=H4+H12, XE3=H8 (sums of existing one-hot
  masks), Dm=1[w=-m] (negated on-chip).

All matmul operands are fp16 (0/1 masks, small ints, 1536+-8: all exact);
PSUM accumulates in fp32 and every partial sum stays < 2^24, so the whole
pipeline is integer-exact.

Sharding: D (contraction, 1024) split across 8 cores, 128 rows each; every
core computes a full [64, 1024] partial in PSUM. Host sums the 8 partials
(exact), adds bias in fp32 and applies relu -- bit-identical to the
reference.
"""

import numpy as np

B, D, U, BITS = 64, 1024, 1024, 8
NCORES = 8
DSH = D // NCORES  # 128 contraction rows per core
MAGIC = 1536.0
OFF = MAGIC - 511.0 / 1024.0

# engine for each G_v pass: "dve" or "act"
G_ENGINE = {v: ("act" if v in (13, 14, 15) else "dve") for v in range(1, 16)}
N_WARMUP_MM = 6  # dummy matmuls during the DMA-in window to warm the PE HAM
TRACE = False

_NC_CACHE = {}


def _build_nc():
    import concourse.bacc as bacc
    import concourse.mybir as mybir
    import concourse.tile as tile

    Alu = mybir.AluOpType
    f16 = mybir.dt.float16
    i16 = mybir.dt.int16
    f32 = mybir.dt.float32

    nc = bacc.Bacc("TRN2", target_bir_lowering=False, debug=False)
    xt_d = nc.dram_tensor("xt", [DSH, B], i16, kind="ExternalInput")
    w_d = nc.dram_tensor("w", [DSH, U], i16, kind="ExternalInput")
    out_d = nc.dram_tensor("out", [B, U], f32, kind="ExternalOutput")

    with tile.TileContext(nc) as tc:
        with (
            tc.tile_pool(name="io", bufs=1) as io,
            tc.tile_pool(name="ps", bufs=1, space="PSUM") as ps,
        ):
            xt_sb = io.tile([DSH, B], i16)
            w_sb = io.tile([DSH, U], i16)
            nc.sync.dma_start(xt_sb[:], xt_d[:])
            nc.sync.dma_start(w_sb[:], w_d[:])

            # --- PE warmup: dummy matmuls on memset tiles during DMA-in ---
            warm_l = io.tile([DSH, B], f16, tag="warm_l")
            warm_r = io.tile([DSH, 512], f16, tag="warm_r")
            nc.gpsimd.memset(warm_l[:], 1.0)
            nc.gpsimd.memset(warm_r[:], 1.0)
            off_sb = io.tile([DSH, 1], f32, tag="offsb")
            nc.gpsimd.memset(off_sb[:], OFF)
            warm_ps = ps.tile([B, 512], f32, tag="warm_ps")
            for _ in range(N_WARMUP_MM):
                nc.tensor.matmul(
                    warm_ps[:], warm_l[:], warm_r[:], start=True, stop=True
                )

            # --- per-v: one-hot mask (fp16 [DSH, B]), LUT, 2 matmuls ---
            # G_v = 1536 + floor(v*W/256), fp16, one op per v
            acc = ps.tile([B, U], f32)
            h, g = {}, {}
            mm_queue = []  # (lhsT, rhs_tile, half) in PE issue order

            def emit_hv(v):
                t = io.tile([DSH, B], f16, tag=f"h{v}")
                nc.vector.tensor_scalar(
                    out=t[:], in0=xt_sb[:], scalar1=float(v), scalar2=None,
                    op0=Alu.is_equal,
                )
                h[v] = t

            def emit_gv(v):
                t = io.tile([DSH, U], f16, tag=f"g{v}")
                if G_ENGINE[v] == "act":
                    nc.scalar.activation(
                        t[:], w_sb[:], mybir.ActivationFunctionType.Identity,
                        bias=off_sb[:], scale=float(v) / 256.0,
                    )
                else:
                    nc.vector.tensor_scalar(
                        out=t[:], in0=w_sb[:], scalar1=float(v) / 256.0,
                        scalar2=OFF, op0=Alu.mult, op1=Alu.add,
                    )
                g[v] = t

            # --- W-side correction masks (DVE; gpsimd is ~50x slower) ---
            mneg = io.tile([DSH, U], f16, tag="mneg")
            nc.vector.tensor_scalar(
                out=mneg[:], in0=w_sb[:], scalar1=0.0, scalar2=-MAGIC,
                op0=Alu.is_lt, op1=Alu.add,
            )
            dmask = {}
            for m in (128, 64, 32, 96):
                t = io.tile([DSH, U], f16, tag=f"d{m}")
                nc.vector.tensor_scalar(
                    out=t[:], in0=w_sb[:], scalar1=float(-m),
                    scalar2=-1.0, op0=Alu.is_equal, op1=Alu.mult,
                )
                dmask[m] = t

            # ACT-assigned G's first (ACT is otherwise idle; starts at W-DMA)
            for v in range(1, 16):
                if G_ENGINE[v] == "act":
                    emit_gv(v)

            # xnz early on DVE (tiny)
            xnz = io.tile([DSH, B], f16, tag="xnz")
            nc.vector.tensor_scalar(
                out=xnz[:], in0=xt_sb[:], scalar1=1.0, scalar2=None,
                op0=Alu.is_ge,
            )
            for v in range(1, 16):
                emit_hv(v)
                if G_ENGINE[v] != "act":
                    emit_gv(v)
            # XE sums of one-hot masks (disjoint -> still 0/1) + D-mask sums
            xe1 = io.tile([DSH, B], f16, tag="xe1")
            xe1b = io.tile([DSH, B], f16, tag="xe1b")
            xe2 = io.tile([DSH, B], f16, tag="xe2")
            nc.vector.tensor_tensor(
                out=xe1b[:], in0=h[2][:], in1=h[6][:], op=Alu.add
            )
            nc.vector.tensor_tensor(
                out=xe1[:], in0=h[10][:], in1=h[14][:], op=Alu.add
            )
            nc.vector.tensor_tensor(
                out=xe1[:], in0=xe1[:], in1=xe1b[:], op=Alu.add
            )
            nc.vector.tensor_tensor(
                out=xe2[:], in0=h[4][:], in1=h[12][:], op=Alu.add
            )
            p2n = io.tile([DSH, U], f16, tag="p2n")
            nc.vector.tensor_tensor(
                out=p2n[:], in0=dmask[64][:], in1=dmask[128][:], op=Alu.add
            )
            p3n = io.tile([DSH, U], f16, tag="p3n")
            nc.vector.tensor_tensor(
                out=p3n[:], in0=dmask[32][:], in1=dmask[96][:], op=Alu.add
            )
            nc.vector.tensor_tensor(
                out=p3n[:], in0=p3n[:], in1=p2n[:], op=Alu.add
            )

            # --- matmuls: 19 groups x 2 halves; psum [64, 1024] = 2 banks ---
            # v-major pairs for v=1..12 (PE chases DVE), then the remaining
            # 7 groups half-major so bank 0's stop lands ~7 MMs early and
            # its copy+DMA overlap bank 1's matmuls.
            groups = [(h[v], g[v]) for v in range(1, 16)] + [
                (xnz, mneg), (xe1, dmask[128]), (xe2, p2n), (h[8], p3n)]
            n_g = len(groups)
            sched = []  # (gidx, half)
            for gidx in range(12):
                sched += [(gidx, 0), (gidx, 1)]
            sched += [(gidx, 0) for gidx in range(12, n_g)]
            sched += [(gidx, 1) for gidx in range(12, n_g)]
            for gidx, half in sched:
                lhsT, rhs = groups[gidx]
                sl = slice(half * 512, (half + 1) * 512)
                nc.tensor.matmul(
                    acc[:, sl], lhsT[:], rhs[:, sl],
                    start=(gidx == 0), stop=(gidx == n_g - 1),
                )

            # --- epilogue: PSUM -> SBUF fp32 per half, 2 DMAs out ---
            o_sb = io.tile([B, U], f32, tag="osb")
            nc.scalar.copy(o_sb[:, 0:512], acc[:, 0:512])
            nc.sync.dma_start(out_d[:, 0:512], o_sb[:, 0:512])
            nc.scalar.copy(o_sb[:, 512:768], acc[:, 512:768])
            nc.vector.tensor_copy(o_sb[:, 768:1024], acc[:, 768:1024])
            nc.sync.dma_start(out_d[:, 512:1024], o_sb[:, 512:1024])

    nc.compile()
    return nc


def _get_nc():
    if "nc" not in _NC_CACHE:
        _NC_CACHE["nc"] = _build_nc()
    return _NC_CACHE["nc"]


_LAST_RESULTS = {}


def _kernel_numpy(inputs, bits, kernel, bias):
    # generic (non-8-bit) fallback; mirrors the reference exactly
    x = np.asarray(inputs, np.float64)
    w = np.asarray(kernel, np.float64)
    b = int(bits)
    out = np.zeros((x.shape[0], w.shape[1]), np.float64)
    scale = float(2 ** b)
    for d0 in range(0, w.shape[0], 128):
        d1 = min(d0 + 128, w.shape[0])
        wm = np.sign(w[None, d0:d1, :]) * (
            np.abs(w[None, d0:d1, :]) % scale if b < 31 else np.abs(w[None, d0:d1, :])
        )
        out += np.trunc(x[:, d0:d1, None] * wm / scale).sum(1)
    return np.maximum(out + np.asarray(bias, np.float64)[None, :], 0.0).astype(
        np.float32
    )


def kernel(inputs, bits, kernel, bias):
    if int(bits) != BITS:
        return _kernel_numpy(inputs, bits, kernel, bias)

    from concourse.bass_utils import run_bass_kernel_spmd

    x = np.asarray(inputs)
    w = np.asarray(kernel)
    b = np.asarray(bias, dtype=np.float32)
    assert x.shape == (B, D) and w.shape == (D, U)

    xt = np.ascontiguousarray(x.T.astype(np.int16))  # [D, B]
    wi = np.ascontiguousarray(w.astype(np.int16))    # [D, U]

    in_maps = [
        {
            "xt": np.ascontiguousarray(xt[c * DSH:(c + 1) * DSH]),
            "w": np.ascontiguousarray(wi[c * DSH:(c + 1) * DSH]),
        }
        for c in range(NCORES)
    ]

    nc = _get_nc()
    res = run_bass_kernel_spmd(
        nc, in_maps, core_ids=list(range(NCORES)), trace=TRACE
    )
    _LAST_RESULTS["res"] = res

    total = np.zeros((B, U), dtype=np.float32)
    for r in res.results:
        total += r["out"]
    return np.maximum(total + b[None, :], 0.0).astype(np.float32)
